# revision 1
# baseline (speedup 1.0000x reference)
"""Trainium2 Bass kernel: per-pixel 5x5 kernel application (KPN-style).

    out[b,c,y,x] = sum_{i,j} softmax(kernels[b,:,y,x])[i*5+j]
                   * zpad(data)[b,c,y+i,x+j]          (i,j in 0..4, r=2)

Sharding (8 NeuronCores, pure data parallel, no collectives):
    core = (b, H-half): 4 batches x 2 row-bands of 360 rows.
    Halo rows come from overlapping host-side slices of the full input.

Per-core algorithm (all tiles live in "data space": 124 partitions =
120 output rows + 2 halo rows each side):
    - unnormalized softmax: E_t = exp(K_t) on ACT (bf16 out), the
      denominator sum(E_t) is accumulated on the PE together with the taps.
    - DVE forms per-tap product planes Q_{t,c} = E_t * D_c (bf16, 2x mode);
      the x-shift dj is a free-dim offset (two parity-aligned bf16 copies
      of the data keep operands 4-byte aligned), the y-shift di is folded
      into the kernel-tensor DMA (rows loaded shifted by -di) and undone
      by the PE's stationary shift matrix S_di[k,m] = [k == m+di].
    - PE accumulates the 25 tap planes (and the 25 exp planes) into PSUM
      with shifted-identity matmuls: PSUM[m,x] += sum_k S_di[k,m] Q[k,x].
    - DVE: out_c = PSUM_c * reciprocal(PSUM_sum).

kernel(**inputs) takes the FULL inputs and returns the FULL output.
"""

import os
import numpy as np
import ml_dtypes

B, C, H, W, KW = 4, 3, 720, 1280, 5
NCORES = 8
HS = H // 2            # 360 output rows per shard
RT = 120               # output rows per row-tile
NRT = HS // RT         # 3 row-tiles
HALO = 2
DP = RT + 2 * HALO     # 124 partitions (data space)
WP = 1288              # padded data width: 2 left + 1280 + 6 right
KROWPAD = 4            # zero rows around each kernel shard (top+bottom)
KH = HS + 2 * KROWPAD  # 368
XCH = [(0, 512), (512, 512), (1024, 256)]

_CACHE = {}


def _build_program():
    import concourse.bacc as bacc
    import concourse.mybir as mybir
    from concourse.bass import AP
    from concourse import tile

    f32 = mybir.dt.float32
    bf16 = mybir.dt.bfloat16

    nc = bacc.Bacc(
        "TRN2",
        target_bir_lowering=False,
        debug=False,
        enable_asserts=False,
        num_devices=NCORES,
    )
    d_data = nc.dram_tensor("data", [C, HS + 2 * HALO, WP], f32, kind="ExternalInput")
    d_kern = nc.dram_tensor("kern", [KW * KW, KH, W], f32, kind="ExternalInput")
    d_out = nc.dram_tensor("out", [C, HS, W], f32, kind="ExternalOutput")

    # Shift matrices S_di[k, m] = 1 iff k == m + di  (k: 124 data rows,
    # m: 120 out rows). Baked into the NEFF as a Const tensor.
    s_np = np.zeros((KW, DP, RT), dtype=ml_dtypes.bfloat16)
    for di in range(KW):
        for m in range(RT):
            s_np[di, m + di, m] = 1.0
    d_s = nc.inline_tensor(np.ascontiguousarray(s_np), "smat")

    KSTR_T = KH * W  # element stride between taps of d_kern

    with tile.TileContext(nc) as tc:
        with tc.tile_pool(name="const", bufs=1) as cpool, \
             tc.tile_pool(name="dd", bufs=2) as dpool, \
             tc.tile_pool(name="dbf", bufs=2) as dbfpool, \
             tc.tile_pool(name="kt", bufs=3) as kpool, \
             tc.tile_pool(name="et", bufs=3) as epool, \
             tc.tile_pool(name="qt", bufs=6) as qpool, \
             tc.tile_pool(name="fin", bufs=2) as fpool, \
             tc.tile_pool(name="ps", bufs=2, space="PSUM") as ppool:

            s_sb = cpool.tile([DP, KW, RT], bf16)
            nc.sync.dma_start(out=s_sb[:], in_=d_s.ap().transpose([1, 0, 2]))

            for rt in range(NRT):
                y0 = rt * RT

                # data rows y0 .. y0+123 of the (row-padded) shard
                ddat = dpool.tile([DP, C, WP], f32, tag="ddat")
                nc.sync.dma_start(
                    out=ddat[:],
                    in_=d_data.ap().transpose([1, 0, 2])[y0:y0 + DP],
                )
                # bf16 copies; dbf1 shifted one element left so odd dj
                # slices stay 4-byte aligned (keeps DVE 2x mode).
                dbf0 = dbfpool.tile([DP, C, WP], bf16, tag="dbf0")
                nc.vector.tensor_copy(dbf0[:], ddat[:])
                dbf1 = dbfpool.tile([DP, C, WP], bf16, tag="dbf1")
                nc.sync.dma_start(out=dbf1[:, :, 0:WP - 1], in_=dbf0[:, :, 1:WP])

                for (xc, xcw) in XCH:
                    # PSUM banks: 0..2 = channel accumulators, 3 = sumexp
                    pacc = ppool.tile([RT, 4, 512], f32, tag="pacc")

                    for di in range(KW):
                        # kt[p, dj, x] = kern[di*5+dj, KROWPAD+y0+p-di, xc+x]
                        kt = kpool.tile([DP, KW, 512], f32, tag="kt")
                        off = di * KW * KSTR_T + (KROWPAD + y0 - di) * W + xc
                        nc.sync.dma_start(
                            out=kt[:, :, 0:xcw],
                            in_=AP(d_kern, off, [[W, DP], [KSTR_T, KW], [1, xcw]]),
                        )
                        et = epool.tile([DP, KW, 512], bf16, tag="et")
                        nc.scalar.activation(
                            et[:, :, 0:xcw], kt[:, :, 0:xcw],
                            mybir.ActivationFunctionType.Exp,
                        )

                        lhs = s_sb[:, di, :]
                        first = di == 0
                        last = di == KW - 1
                        for dj in range(KW):
                            nc.tensor.matmul(
                                out=pacc[:, 3, 0:xcw],
                                lhsT=lhs,
                                rhs=et[:, dj, 0:xcw],
                                start=first and dj == 0,
                                stop=last and dj == KW - 1,
                            )
                        for dj in range(KW):
                            qt = qpool.tile([DP, C, 512], bf16, tag="qt")
                            if dj % 2 == 0:
                                dsrc = dbf0[:, :, xc + dj:xc + dj + xcw]
                            else:
                                dsrc = dbf1[:, :, xc + dj - 1:xc + dj - 1 + xcw]
                            esrc = et[:, dj, 0:xcw].unsqueeze(1).broadcast_to([DP, C, xcw])
                            nc.vector.tensor_tensor(
                                qt[:, :, 0:xcw], esrc, dsrc, mybir.AluOpType.mult,
                            )
                            for c in range(C):
                                nc.tensor.matmul(
                                    out=pacc[:, c, 0:xcw],
                                    lhsT=lhs,
                                    rhs=qt[:, c, 0:xcw],
                                    start=first and dj == 0,
                                    stop=last and dj == KW - 1,
                                )

                    rs = fpool.tile([RT, 512], f32, tag="rs")
                    nc.vector.reciprocal(rs[:, 0:xcw], pacc[:, 3, 0:xcw])
                    ost = fpool.tile([RT, C, 512], f32, tag="ost")
                    rsb = rs[:, 0:xcw].unsqueeze(1).broadcast_to([RT, C, xcw])
                    nc.vector.tensor_tensor(
                        ost[:, :, 0:xcw], pacc[:, 0:3, 0:xcw], rsb,
                        mybir.AluOpType.mult,
                    )
                    nc.sync.dma_start(
                        out=d_out.ap().transpose([1, 0, 2])[y0:y0 + RT, :, xc:xc + xcw],
                        in_=ost[:, :, 0:xcw],
                    )

    nc.compile()
    return nc


def get_program():
    if "nc" not in _CACHE:
        _CACHE["nc"] = _build_program()
    return _CACHE["nc"]


def make_shards(data: np.ndarray, kernels: np.ndarray):
    """Full inputs -> per-core input maps (with halo + zero padding)."""
    data = np.asarray(data, dtype=np.float32)
    kernels = np.asarray(kernels, dtype=np.float32)
    # zero-pad data: 2 rows top/bottom, 2 cols left, 6 cols right
    dpad = np.zeros((B, C, H + 2 * HALO, WP), dtype=np.float32)
    dpad[:, :, HALO:HALO + H, HALO:HALO + W] = data
    in_maps = []
    for core in range(NCORES):
        b, hh = divmod(core, 2)
        r0 = hh * HS
        dsh = np.ascontiguousarray(dpad[b, :, r0:r0 + HS + 2 * HALO, :])
        ksh = np.zeros((KW * KW, KH, W), dtype=np.float32)
        ksh[:, KROWPAD:KROWPAD + HS, :] = kernels[b, :, r0:r0 + HS, :]
        in_maps.append({"data": dsh, "kern": ksh})
    return in_maps


def assemble(results) -> np.ndarray:
    out = np.empty((B, C, H, W), dtype=np.float32)
    for core in range(NCORES):
        b, hh = divmod(core, 2)
        out[b, :, hh * HS:(hh + 1) * HS, :] = results[core]["out"]
    return out


def kernel(data: np.ndarray, kernels: np.ndarray) -> np.ndarray:
    from concourse.bass_utils import run_bass_kernel_spmd

    nc = get_program()
    in_maps = make_shards(data, kernels)
    res = run_bass_kernel_spmd(nc, in_maps, list(range(NCORES)))
    return assemble(res.results)


if __name__ == "__main__":
    # smoke test: build only
    get_program()
    print("program built OK")


# revision 2
# speedup vs baseline: 1.5621x; 1.5621x over previous
"""Trainium2 Bass kernel: per-pixel 5x5 kernel application (KPN-style).

    out[b,c,y,x] = sum_{i,j} softmax(kernels[b,:,y,x])[i*5+j]
                   * zpad(data)[b,c,y+i,x+j]          (i,j in 0..4, r=2)

Sharding (8 NeuronCores, pure data parallel, no collectives):
    core = (b, H-half): 4 batches x 2 row-bands of 360 rows.
    Halo rows come from overlapping host-side slices of the full input.

Per-core algorithm (tiles live in "data space": 124 partitions =
120 output rows + 2 halo rows each side):
    - unnormalized softmax: E_t = exp(K_t) on ACT (bf16), denominator
      accumulated on the PE together with the taps.
    - DVE forms per-tap product planes Q_{t,c} = E_t * D_c (bf16, 2x mode);
      x-shift dj is a free-dim offset (two parity-aligned bf16 copies of the
      data keep operands 4-byte aligned); y-shift di is folded into the
      kernel-tensor DMA (rows loaded shifted by -di) and undone by the PE's
      stationary shift matrix S_di[k,m] = [k == m+di].
    - PE accumulates the 25 tap planes (and the 25 exp planes) into PSUM
      with shifted-identity matmuls.
    - DVE: out_c = PSUM_c * reciprocal(PSUM_sum).

DMA layout notes: kernel-tensor loads are one DMA per tap plane so the
DRAM side is a single contiguous region (strided small-chunk patterns get
pinned to a few SDMA engines); inputs are pre-converted to bf16 on the
host to halve HBM traffic; loads alternate between the two HWDGE rings
(sync + scalar); the output is staged full-width and stored once per
row-tile.

kernel(**inputs) takes the FULL inputs and returns the FULL output.
"""

import numpy as np
import ml_dtypes

B, C, H, W, KW = 4, 3, 720, 1280, 5
NCORES = 8
HS = H // 2            # 360 output rows per shard
RT = 120               # output rows per row-tile
NRT = HS // RT         # 3 row-tiles
HALO = 2
DP = RT + 2 * HALO     # 124 partitions (data space)
WP = 1288              # padded data width: 2 left + 1280 + 6 right
KROWPAD = 4            # zero rows around each kernel shard (top+bottom)
KH = HS + 2 * KROWPAD  # 368
XCH = [(0, 512), (512, 512), (1024, 256)]

KERN_BF16 = True       # ship kernels to HBM as bf16 (halves DMA traffic)

_CACHE = {}


def _build_program():
    import concourse.bacc as bacc
    import concourse.mybir as mybir
    from concourse.bass import AP
    from concourse import tile

    f32 = mybir.dt.float32
    bf16 = mybir.dt.bfloat16
    kdt = bf16 if KERN_BF16 else f32

    nc = bacc.Bacc(
        "TRN2",
        target_bir_lowering=False,
        debug=False,
        enable_asserts=False,
        num_devices=NCORES,
    )
    d_data = nc.dram_tensor("data", [C, HS + 2 * HALO, WP], bf16, kind="ExternalInput")
    d_kern = nc.dram_tensor("kern", [KW * KW, KH, W], kdt, kind="ExternalInput")
    d_out = nc.dram_tensor("out", [C, HS, W], f32, kind="ExternalOutput")

    # Shift matrices S_di[k, m] = 1 iff k == m + di  (k: 124 data rows,
    # m: 120 out rows). Baked into the NEFF as a Const tensor.
    s_np = np.zeros((KW, DP, RT), dtype=ml_dtypes.bfloat16)
    for di in range(KW):
        for m in range(RT):
            s_np[di, m + di, m] = 1.0
    d_s = nc.inline_tensor(np.ascontiguousarray(s_np), "smat")

    KSTR_T = KH * W  # element stride between taps of d_kern

    with tile.TileContext(nc) as tc:
        with tc.tile_pool(name="const", bufs=1) as cpool, \
             tc.tile_pool(name="dbf", bufs=2) as dbfpool, \
             tc.tile_pool(name="kt", bufs=2) as kpool, \
             tc.tile_pool(name="et", bufs=6) as epool, \
             tc.tile_pool(name="qt", bufs=5) as qpool, \
             tc.tile_pool(name="fin", bufs=2) as fpool, \
             tc.tile_pool(name="ps", bufs=2, space="PSUM") as ppool:

            s_sb = cpool.tile([DP, KW, RT], bf16)
            nc.sync.dma_start(out=s_sb[:], in_=d_s.ap().transpose([1, 0, 2]))

            for rt in range(NRT):
                y0 = rt * RT

                # data rows y0 .. y0+123 of the (row-padded) shard, bf16.
                # dbf1 is shifted one element left so odd-dj slices stay
                # 4-byte aligned (keeps DVE 2x mode).
                dbf0 = dbfpool.tile([DP, C, WP], bf16, tag="dbf0")
                nc.scalar.dma_start(
                    out=dbf0[:],
                    in_=d_data.ap().transpose([1, 0, 2])[y0:y0 + DP],
                )
                dbf1 = dbfpool.tile([DP, C, WP], bf16, tag="dbf1")
                nc.sync.dma_start(out=dbf1[:, :, 0:WP - 1], in_=dbf0[:, :, 1:WP])

                # kernel taps: one contiguous full-width DMA per tap plane,
                # rows shifted by -di; exp per di-group.
                ets = []
                for di in range(KW):
                    kt = kpool.tile([DP, KW, W], kdt, tag="kt")
                    for dj in range(KW):
                        t = di * KW + dj
                        off = t * KSTR_T + (KROWPAD + y0 - di) * W
                        eng = nc.sync if t % 2 == 0 else nc.scalar
                        eng.dma_start(
                            out=kt[:, dj, :],
                            in_=AP(d_kern, off, [[W, DP], [1, W]]),
                        )
                    et = epool.tile([DP, KW, W], bf16, tag="et")
                    nc.scalar.activation(
                        et[:], kt[:], mybir.ActivationFunctionType.Exp,
                    )
                    ets.append(et)

                rs = fpool.tile([RT, W], f32, tag="rs")
                ost = fpool.tile([RT, C, W], f32, tag="ost")

                for (xc, xcw) in XCH:
                    # PSUM banks: 0..2 = channel accumulators, 3 = sumexp
                    pacc = ppool.tile([RT, 4, 512], f32, tag="pacc")

                    for di in range(KW):
                        et = ets[di]
                        lhs = s_sb[:, di, :]
                        first = di == 0
                        last = di == KW - 1
                        for dj in range(KW):
                            nc.tensor.matmul(
                                out=pacc[:, 3, 0:xcw],
                                lhsT=lhs,
                                rhs=et[:, dj, xc:xc + xcw],
                                start=first and dj == 0,
                                stop=last and dj == KW - 1,
                            )
                        for dj in range(KW):
                            qt = qpool.tile([DP, C, 512], bf16, tag="qt")
                            if dj % 2 == 0:
                                dsrc = dbf0[:, :, xc + dj:xc + dj + xcw]
                            else:
                                dsrc = dbf1[:, :, xc + dj - 1:xc + dj - 1 + xcw]
                            esrc = (
                                et[:, dj, xc:xc + xcw]
                                .unsqueeze(1)
                                .broadcast_to([DP, C, xcw])
                            )
                            nc.vector.tensor_tensor(
                                qt[:, :, 0:xcw], esrc, dsrc, mybir.AluOpType.mult,
                            )
                            for c in range(C):
                                nc.tensor.matmul(
                                    out=pacc[:, c, 0:xcw],
                                    lhsT=lhs,
                                    rhs=qt[:, c, 0:xcw],
                                    start=first and dj == 0,
                                    stop=last and dj == KW - 1,
                                )

                    nc.vector.reciprocal(rs[:, xc:xc + xcw], pacc[:, 3, 0:xcw])
                    rsb = (
                        rs[:, xc:xc + xcw].unsqueeze(1).broadcast_to([RT, C, xcw])
                    )
                    nc.vector.tensor_tensor(
                        ost[:, :, xc:xc + xcw], pacc[:, 0:3, 0:xcw], rsb,
                        mybir.AluOpType.mult,
                    )

                nc.gpsimd.dma_start(
                    out=d_out.ap().transpose([1, 0, 2])[y0:y0 + RT],
                    in_=ost[:],
                )

    nc.compile()
    return nc


def get_program():
    if "nc" not in _CACHE:
        _CACHE["nc"] = _build_program()
    return _CACHE["nc"]


def make_shards(data: np.ndarray, kernels: np.ndarray):
    """Full inputs -> per-core input maps (with halo + zero padding)."""
    data = np.asarray(data, dtype=np.float32)
    kernels = np.asarray(kernels, dtype=np.float32)
    kdt = ml_dtypes.bfloat16 if KERN_BF16 else np.float32
    # zero-pad data: 2 rows top/bottom, 2 cols left, 6 cols right
    dpad = np.zeros((B, C, H + 2 * HALO, WP), dtype=ml_dtypes.bfloat16)
    dpad[:, :, HALO:HALO + H, HALO:HALO + W] = data.astype(ml_dtypes.bfloat16)
    in_maps = []
    for core in range(NCORES):
        b, hh = divmod(core, 2)
        r0 = hh * HS
        dsh = np.ascontiguousarray(dpad[b, :, r0:r0 + HS + 2 * HALO, :])
        ksh = np.zeros((KW * KW, KH, W), dtype=kdt)
        ksh[:, KROWPAD:KROWPAD + HS, :] = kernels[b, :, r0:r0 + HS, :].astype(kdt)
        in_maps.append({"data": dsh, "kern": ksh})
    return in_maps


def assemble(results) -> np.ndarray:
    out = np.empty((B, C, H, W), dtype=np.float32)
    for core in range(NCORES):
        b, hh = divmod(core, 2)
        out[b, :, hh * HS:(hh + 1) * HS, :] = results[core]["out"]
    return out


def kernel(data: np.ndarray, kernels: np.ndarray) -> np.ndarray:
    from concourse.bass_utils import run_bass_kernel_spmd

    nc = get_program()
    in_maps = make_shards(data, kernels)
    res = run_bass_kernel_spmd(nc, in_maps, list(range(NCORES)))
    return assemble(res.results)


if __name__ == "__main__":
    get_program()
    print("program built OK")


# revision 4
# speedup vs baseline: 1.6563x; 1.0603x over previous
"""Trainium2 Bass kernel: per-pixel 5x5 kernel application (KPN-style).

    out[b,c,y,x] = sum_{i,j} softmax(kernels[b,:,y,x])[i*5+j]
                   * zpad(data)[b,c,y+i,x+j]          (i,j in 0..4, r=2)

Sharding (8 NeuronCores, pure data parallel, no collectives):
    core = (b, H-half): 4 batches x 2 row-bands of 360 rows.
    Halo rows come from overlapping host-side slices of the full input.

Per-core algorithm (tiles live in "data space": 124 partitions =
120 output rows + 2 halo rows each side):
    - unnormalized softmax: E_t = exp(K_t) on ACT (bf16), denominator
      accumulated on the PE together with the taps.
    - DVE forms per-tap product planes Q_{t,c} = E_t * D_c (bf16, 2x mode);
      x-shift dj is a free-dim offset (two parity-aligned bf16 copies of the
      data keep operands 4-byte aligned); y-shift di is folded into the
      kernel-tensor DMA (rows loaded shifted by -di) and undone by the PE's
      stationary shift matrix S_di[k,m] = [k == m+di].
    - PE accumulates the 25 tap planes (and the 25 exp planes) into PSUM
      with shifted-identity matmuls.
    - DVE: out_c = PSUM_c * reciprocal(PSUM_sum).

DMA layout notes: kernel-tensor loads are one DMA per tap plane so the
DRAM side is a single contiguous region (strided small-chunk patterns get
pinned to a few SDMA engines); inputs are pre-converted to bf16 on the
host to halve HBM traffic; loads alternate between the two HWDGE rings
(sync + scalar); the output is staged full-width and stored once per
row-tile.

kernel(**inputs) takes the FULL inputs and returns the FULL output.
"""

import numpy as np
import ml_dtypes

B, C, H, W, KW = 4, 3, 720, 1280, 5
NCORES = 8
HS = H // 2            # 360 output rows per shard
RT = 120               # output rows per row-tile
NRT = HS // RT         # 3 row-tiles
HALO = 2
DP = RT + 2 * HALO     # 124 partitions (data space)
WP = 1288              # padded data width: 2 left + 1280 + 6 right
KROWPAD = 4            # zero rows around each kernel shard (top+bottom)
KH = HS + 2 * KROWPAD  # 368
XCH = [(0, 512), (512, 512), (1024, 256)]

KERN_BF16 = True       # ship kernels to HBM as bf16 (halves DMA traffic)

_CACHE = {}


def _build_program():
    import concourse.bacc as bacc
    import concourse.mybir as mybir
    from concourse.bass import AP
    from concourse import tile

    f32 = mybir.dt.float32
    bf16 = mybir.dt.bfloat16
    kdt = bf16 if KERN_BF16 else f32

    nc = bacc.Bacc(
        "TRN2",
        target_bir_lowering=False,
        debug=False,
        enable_asserts=False,
        num_devices=NCORES,
    )
    d_data = nc.dram_tensor("data", [C, HS + 2 * HALO, WP], bf16, kind="ExternalInput")
    d_kern = nc.dram_tensor("kern", [KW * KW, KH, W], kdt, kind="ExternalInput")
    d_out = nc.dram_tensor("out", [C, HS, W], f32, kind="ExternalOutput")

    # Shift matrices S_di[k, m] = 1 iff k == m + di  (k: 124 data rows,
    # m: 120 out rows). Baked into the NEFF as a Const tensor.
    s_np = np.zeros((KW, DP, RT), dtype=ml_dtypes.bfloat16)
    for di in range(KW):
        for m in range(RT):
            s_np[di, m + di, m] = 1.0
    d_s = nc.inline_tensor(np.ascontiguousarray(s_np), "smat")

    KSTR_T = KH * W  # element stride between taps of d_kern

    with tile.TileContext(nc) as tc:
        with tc.tile_pool(name="const", bufs=1) as cpool, \
             tc.tile_pool(name="dbf", bufs=2) as dbfpool, \
             tc.tile_pool(name="kt", bufs=3) as kpool, \
             tc.tile_pool(name="et", bufs=6) as epool, \
             tc.tile_pool(name="qt", bufs=5) as qpool, \
             tc.tile_pool(name="fin", bufs=2) as fpool, \
             tc.tile_pool(name="ps", bufs=2, space="PSUM") as ppool:

            s_sb = cpool.tile([DP, KW, RT], bf16)
            nc.sync.dma_start(out=s_sb[:], in_=d_s.ap().transpose([1, 0, 2]))

            for rt in range(NRT):
                y0 = rt * RT

                # data rows y0 .. y0+123 of the (row-padded) shard, bf16.
                # dbf1 is shifted one element left so odd-dj slices stay
                # 4-byte aligned (keeps DVE 2x mode).
                dbf0 = dbfpool.tile([DP, C, WP], bf16, tag="dbf0")
                nc.scalar.dma_start(
                    out=dbf0[:],
                    in_=d_data.ap().transpose([1, 0, 2])[y0:y0 + DP],
                )
                dbf1 = dbfpool.tile([DP, C, WP], bf16, tag="dbf1")
                nc.sync.dma_start(out=dbf1[:, :, 0:WP - 1], in_=dbf0[:, :, 1:WP])

                # kernel taps: one SWDGE DMA per di-group (5 tap planes,
                # rows shifted by -di) — SWDGE spreads descriptors across
                # all 16 SDMA engines (HWDGE pins them to 4); exp per group.
                ets = []
                for di in range(KW):
                    kt = kpool.tile([DP, KW, W], kdt, tag="kt")
                    off = di * KW * KSTR_T + (KROWPAD + y0 - di) * W
                    nc.gpsimd.dma_start(
                        out=kt[:],
                        in_=AP(d_kern, off, [[W, DP], [KSTR_T, KW], [1, W]]),
                    )
                    et = epool.tile([DP, KW, W], bf16, tag="et")
                    nc.scalar.activation(
                        et[:], kt[:], mybir.ActivationFunctionType.Exp,
                    )
                    ets.append(et)

                rs = fpool.tile([RT, W], f32, tag="rs")
                ost = fpool.tile([RT, C, W], f32, tag="ost")

                for (xc, xcw) in XCH:
                    # PSUM banks: 0..2 = channel accumulators, 3 = sumexp
                    pacc = ppool.tile([RT, 4, 512], f32, tag="pacc")

                    for di in range(KW):
                        et = ets[di]
                        lhs = s_sb[:, di, :]
                        first = di == 0
                        last = di == KW - 1
                        for dj in range(KW):
                            nc.tensor.matmul(
                                out=pacc[:, 3, 0:xcw],
                                lhsT=lhs,
                                rhs=et[:, dj, xc:xc + xcw],
                                start=first and dj == 0,
                                stop=last and dj == KW - 1,
                            )
                        for dj in range(KW):
                            qt = qpool.tile([DP, C, 512], bf16, tag="qt")
                            if dj % 2 == 0:
                                dsrc = dbf0[:, :, xc + dj:xc + dj + xcw]
                            else:
                                dsrc = dbf1[:, :, xc + dj - 1:xc + dj - 1 + xcw]
                            esrc = (
                                et[:, dj, xc:xc + xcw]
                                .unsqueeze(1)
                                .broadcast_to([DP, C, xcw])
                            )
                            nc.vector.tensor_tensor(
                                qt[:, :, 0:xcw], esrc, dsrc, mybir.AluOpType.mult,
                            )
                            for c in range(C):
                                nc.tensor.matmul(
                                    out=pacc[:, c, 0:xcw],
                                    lhsT=lhs,
                                    rhs=qt[:, c, 0:xcw],
                                    start=first and dj == 0,
                                    stop=last and dj == KW - 1,
                                )

                    nc.vector.reciprocal(rs[:, xc:xc + xcw], pacc[:, 3, 0:xcw])
                    rsb = (
                        rs[:, xc:xc + xcw].unsqueeze(1).broadcast_to([RT, C, xcw])
                    )
                    nc.vector.tensor_tensor(
                        ost[:, :, xc:xc + xcw], pacc[:, 0:3, 0:xcw], rsb,
                        mybir.AluOpType.mult,
                    )

                nc.gpsimd.dma_start(
                    out=d_out.ap().transpose([1, 0, 2])[y0:y0 + RT],
                    in_=ost[:],
                )

    nc.compile()
    return nc


def get_program():
    if "nc" not in _CACHE:
        _CACHE["nc"] = _build_program()
    return _CACHE["nc"]


def make_shards(data: np.ndarray, kernels: np.ndarray):
    """Full inputs -> per-core input maps (with halo + zero padding)."""
    data = np.asarray(data, dtype=np.float32)
    kernels = np.asarray(kernels, dtype=np.float32)
    kdt = ml_dtypes.bfloat16 if KERN_BF16 else np.float32
    # zero-pad data: 2 rows top/bottom, 2 cols left, 6 cols right
    dpad = np.zeros((B, C, H + 2 * HALO, WP), dtype=ml_dtypes.bfloat16)
    dpad[:, :, HALO:HALO + H, HALO:HALO + W] = data.astype(ml_dtypes.bfloat16)
    in_maps = []
    for core in range(NCORES):
        b, hh = divmod(core, 2)
        r0 = hh * HS
        dsh = np.ascontiguousarray(dpad[b, :, r0:r0 + HS + 2 * HALO, :])
        ksh = np.zeros((KW * KW, KH, W), dtype=kdt)
        ksh[:, KROWPAD:KROWPAD + HS, :] = kernels[b, :, r0:r0 + HS, :].astype(kdt)
        in_maps.append({"data": dsh, "kern": ksh})
    return in_maps


def assemble(results) -> np.ndarray:
    out = np.empty((B, C, H, W), dtype=np.float32)
    for core in range(NCORES):
        b, hh = divmod(core, 2)
        out[b, :, hh * HS:(hh + 1) * HS, :] = results[core]["out"]
    return out


def kernel(data: np.ndarray, kernels: np.ndarray) -> np.ndarray:
    from concourse.bass_utils import run_bass_kernel_spmd

    nc = get_program()
    in_maps = make_shards(data, kernels)
    res = run_bass_kernel_spmd(nc, in_maps, list(range(NCORES)))
    return assemble(res.results)


if __name__ == "__main__":
    get_program()
    print("program built OK")
